# revision 12
# baseline (speedup 1.0000x reference)
"""Bayesian multi-head attention on 8 Trainium2 NeuronCores (Bass/Tile).

Sharding: pure data-parallel over batch (B=8 -> 1 batch per core), weights
replicated. Per core, everything is computed in "transposed" layouts so the
matmul contraction dim always sits on SBUF partitions:

  QT/KT  [od, tok]   (W.T stationary over H.T)
  V      [tok, od]   interleaved with per-head ones columns (for softmax sums)
  scores [m, n] per head, PSUM-prefilled with the per-head RBF kernel via an
                 identity matmul so ACT computes exp(scores+rbf) from PSUM
  ctx.T  [d, n]      V-stationary over e-tiles, softmax sum rides aug column
  out    [tok, od]   ctx.T-stationary over Wo.T, final layout, no transposes

Runtime specialization (checked in kernel(), numpy fallback otherwise):
  Wv_rho/Wo_rho/b*_rho are constant -> the variational variance projections
  are rank-1: V_v[m,od] = cv2*hh[m]+cbv with hh=rowsum(H^2), so
  var_attn per head is a single weighted reduction of e^2 and
  out_var = co2*sum_D(ctx_n^2) + cbo + var_attn[head].
"""

import sys
sys.path.insert(0, "/opt/trn_rl_repo")

import numpy as np

P = 128
CH = 512  # matmul free-dim chunk (fp32 psum bank)


def _softplus(x):
    return np.log1p(np.exp(-np.abs(x))) + np.maximum(x, 0.0)


class Cfg:
    def __init__(self, N=1024, D=1024, NH=16, draw=16, shared_rbf=True,
                 inv2l2=None, logs2=None, e2_act_heads=()):
        self.N, self.D, self.NH, self.draw = N, D, NH, draw
        self.DK = D // NH
        assert self.DK == 64, "head pairing layout assumes d_k == 64"
        self.nmb = N // P      # token blocks (key side)
        self.nod = D // P      # feature blocks
        self.nch = max(1, N // CH)   # free chunks over tokens
        self.ndch = max(1, D // CH)  # free chunks over features
        self.chn = min(CH, N)
        self.chd = min(CH, D)
        self.shared_rbf = shared_rbf
        # per-head (or shared) rbf params, baked as immediates
        self.inv2l2 = inv2l2   # shared: float; else list[NH]
        self.logs2 = logs2
        self.e2_act_heads = set(e2_act_heads)  # heads whose e^2 runs on ACT

    def key(self):
        return (self.N, self.D, self.NH, self.draw, self.shared_rbf,
                tuple(np.atleast_1d(self.inv2l2).tolist()),
                tuple(np.atleast_1d(self.logs2).tolist()),
                tuple(sorted(self.e2_act_heads)))


def build_program(cfg: Cfg):
    import concourse.bacc as bacc
    import concourse.mybir as mybir
    import concourse.tile as tile

    f32 = mybir.dt.float32
    Alu = mybir.AluOpType
    Act = mybir.ActivationFunctionType
    N, D, NH, DK, draw = cfg.N, cfg.D, cfg.NH, cfg.DK, cfg.draw
    nmb, nod, nch, ndch, chn, chd = (cfg.nmb, cfg.nod, cfg.nch, cfg.ndch,
                                     cfg.chn, cfg.chd)

    nc = bacc.Bacc("TRN2", target_bir_lowering=False, debug=False,
                   num_devices=8)

    # ---- dram I/O ----
    dt_in = {}
    def din(name, shape):
        dt_in[name] = nc.dram_tensor(name, list(shape), f32,
                                     kind="ExternalInput")
        return dt_in[name]

    HT_d = din("HT", [D, N])
    xT_d = din("xT", [draw, N])
    xTm2_d = din("xTm2", [draw, N])
    x2row_d = din("x2row", [1, N])
    x2col_d = din("x2col", [P, nmb])
    hh2col_d = din("hh2col", [P, nmb])
    WqT_d = din("WqT", [D, D])
    WkT_d = din("WkT", [D, D])
    WvT_d = din("WvT", [D, D])
    WoT_d = din("WoT", [D, D])
    bqc_d = din("bqc", [P, nod])
    bkc_d = din("bkc", [P, nod])
    bvrow_d = din("bvrow", [1, D])
    borow_d = din("borow", [1, D])
    I128_d = din("I128", [P, P])
    selrep_d = din("selrep", [NH, P])
    om_d = nc.dram_tensor("om", [N, D], f32, kind="ExternalOutput")
    ov_d = nc.dram_tensor("ov", [N, D], f32, kind="ExternalOutput")

    ctx_d = nc.dram_tensor("ctx_scratch", [D, N], f32)

    with tile.TileContext(nc) as tc:
        with tc.tile_pool(name="const", bufs=1) as cpool, \
             tc.tile_pool(name="big", bufs=1) as bpool, \
             tc.tile_pool(name="rows", bufs=1) as rpool, \
             tc.tile_pool(name="wstream", bufs=3) as wpool, \
             tc.tile_pool(name="estream", bufs=2) as epool, \
             tc.tile_pool(name="mstream", bufs=2) as mpool, \
             tc.tile_pool(name="psA", bufs=2, space="PSUM") as psA, \
             tc.tile_pool(name="psB", bufs=2, space="PSUM") as psB:

            # ---- constants / small inputs ----
            I128 = cpool.tile([P, P], f32)
            nc.sync.dma_start(I128[:], I128_d.ap())
            selrep = cpool.tile([NH, P], f32)
            nc.sync.dma_start(selrep[:], selrep_d.ap())
            xT = mpool.tile([draw, N], f32, tag="cl")
            nc.sync.dma_start(xT[:], xT_d.ap())
            xTm2 = mpool.tile([draw, N], f32, tag="cl")
            nc.sync.dma_start(xTm2[:], xTm2_d.ap())
            x2row = mpool.tile([1, N], f32, tag="rp", bufs=1)
            nc.sync.dma_start(x2row[:], x2row_d.ap())
            x2col = cpool.tile([P, nmb], f32)
            nc.sync.dma_start(x2col[:], x2col_d.ap())
            hh2col = cpool.tile([P, nmb], f32)
            nc.sync.dma_start(hh2col[:], hh2col_d.ap())
            bqc = cpool.tile([P, nod], f32)
            nc.sync.dma_start(bqc[:], bqc_d.ap())
            bkc = cpool.tile([P, nod], f32)
            nc.sync.dma_start(bkc[:], bkc_d.ap())
            bvrow = mpool.tile([1, D], f32, tag="stg")
            nc.sync.dma_start(bvrow[:], bvrow_d.ap())
            borow = cpool.tile([1, D], f32)
            nc.sync.dma_start(borow[:], borow_d.ap())
            onesrow = cpool.tile([1, P], f32)
            nc.vector.memset(onesrow[:], 1.0)
            onescol = cpool.tile([P, 1], f32)
            nc.vector.memset(onescol[:], 1.0)
            zcol = cpool.tile([P, 1], f32)
            nc.vector.memset(zcol[:], 0.0)

            # ---- big persistent tensors (tag-shared slots) ----
            # slot tH: HT (P1-P2) -> WoT (P3); slot tE: E/d2 (P0-P2) -> ctxNT
            HT = bpool.tile([P, nod, N], f32, tag="tH")
            nc.sync.dma_start(
                HT[:], HT_d.ap().rearrange("(k p) n -> p k n", p=P))
            V = bpool.tile([P, nmb, NH, DK + 1], f32, tag="tV")
            # shared-rbf: E tiles; general: d2 tiles
            Ed2 = bpool.tile([P, nmb, N], f32, tag="tE")

            srows = rpool.tile([NH, N], f32, tag="srows")
            varows = rpool.tile([NH, N], f32)
            rinv = rpool.tile([NH, N], f32)
            rinv2 = rpool.tile([NH, N], f32)
            vaq = rpool.tile([NH + 1, N], f32, tag="srows")

            # ---- P0: rbf groundwork ----
            if cfg.shared_rbf:
                biascol = cpool.tile([P, nmb], f32)
                nc.vector.tensor_scalar(
                    biascol[:], x2col[:], -cfg.inv2l2, cfg.logs2,
                    Alu.mult, Alu.add)
            else:
                logs2col = cpool.tile([P, NH], f32)
                for h in range(NH):
                    nc.vector.memset(logs2col[:, h:h + 1], float(cfg.logs2[h]))

            for m in range(nmb):
                g = psA.tile([P, N], f32, tag="A")
                for c in range(nch):
                    ch = slice(c * chn, (c + 1) * chn)
                    nc.tensor.matmul(g[:, ch], xTm2[:, m * P:(m + 1) * P],
                                     xT[:, ch], start=True, stop=False)
                    nc.tensor.matmul(g[:, ch], onesrow[:], x2row[:, ch],
                                     start=False, stop=True)
                if cfg.shared_rbf:
                    # E = exp(-(d2)/(2 l^2) + log s2);  d2 = g + x2col
                    nc.scalar.activation(Ed2[:, m, :], g[:], Act.Exp,
                                         bias=biascol[:, m:m + 1],
                                         scale=-cfg.inv2l2)
                else:
                    nc.vector.tensor_scalar(Ed2[:, m, :], g[:],
                                            x2col[:, m:m + 1], None, Alu.add)

            # ---- P1: V projection ([tok, od] layout, ones interleaved) ----
            for t in range(nmb):
                pv = psA.tile([P, D], f32, tag="A")
                for c in range(ndch):
                    ch = slice(c * chd, (c + 1) * chd)
                    nc.tensor.matmul(pv[:, ch], onesrow[:], bvrow[:, ch],
                                     start=True, stop=False)
                for k in range(nod):
                    for c in range(ndch):
                        ch = slice(c * chd, (c + 1) * chd)
                        wv = wpool.tile([P, chd], f32, tag="wv")
                        nc.sync.dma_start(
                            wv[:], WvT_d.ap()[k * P:(k + 1) * P, ch])
                        nc.tensor.matmul(pv[:, ch],
                                         HT[:, k, t * P:(t + 1) * P],
                                         wv[:], start=False,
                                         stop=(k == nod - 1))
                nc.vector.tensor_copy(
                    V[:, t, :, 0:DK],
                    pv[:].rearrange("p (h d) -> p h d", h=NH))
                nc.vector.memset(V[:, t, :, DK:DK + 1], 1.0)

            # ---- P2: head pairs (Q/K projected per pair, streamed) ----
            for g in range(NH // 2):
                QT = mpool.tile([P, N], f32, tag="qt")
                KT = mpool.tile([P, N], f32, tag="kt")
                for (Wd, bc, dst) in ((WqT_d, bqc, QT), (WkT_d, bkc, KT)):
                    pp = psA.tile([P, N], f32, tag="A")
                    for k in range(nod):
                        wt = wpool.tile([P, P], f32, tag="w")
                        nc.sync.dma_start(
                            wt[:],
                            Wd.ap()[k * P:(k + 1) * P, g * P:(g + 1) * P])
                        for c in range(nch):
                            ch = slice(c * chn, (c + 1) * chn)
                            nc.tensor.matmul(pp[:, ch], wt[:], HT[:, k, ch],
                                             start=(k == 0),
                                             stop=(k == nod - 1))
                    nc.vector.tensor_scalar(dst[:], pp[:],
                                            bc[:, g:g + 1], None, Alu.add)

                for h in (2 * g, 2 * g + 1):
                    off = (h % 2) * DK
                    C = psB.tile([DK + 1, N], f32, tag="B")
                    R = psB.tile([1, N], f32, tag="B")
                    for m in range(nmb):
                        s = psA.tile([P, N], f32, tag="A")
                        if cfg.shared_rbf:
                            Et = Ed2[:, m, :]
                        else:
                            Et = epool.tile([P, N], f32, tag="rbf")
                            nc.scalar.activation(Et[:], Ed2[:, m, :], Act.Exp,
                                                 bias=logs2col[:, h:h + 1],
                                                 scale=-float(cfg.inv2l2[h]))
                            Et = Et[:]
                        for c in range(nch):
                            ch = slice(c * chn, (c + 1) * chn)
                            nc.tensor.matmul(s[:, ch], I128[:], Et[:, ch],
                                             start=True, stop=False)
                            nc.tensor.matmul(
                                s[:, ch],
                                KT[off:off + DK, m * P:(m + 1) * P],
                                QT[off:off + DK, ch],
                                start=False, stop=True)
                        eT = epool.tile([P, N], f32, tag="e")
                        nc.scalar.activation(eT[:], s[:], Act.Exp,
                                             bias=zcol[:, 0:1])
                        e2T = epool.tile([P, N], f32, tag="e2")
                        if h in cfg.e2_act_heads:
                            nc.scalar.activation(e2T[:], s[:], Act.Exp,
                                                 bias=zcol[:, 0:1], scale=2.0)
                        else:
                            nc.vector.tensor_tensor(e2T[:], eT[:], eT[:],
                                                    Alu.mult)
                        for c in range(nch):
                            ch = slice(c * chn, (c + 1) * chn)
                            nc.tensor.matmul(C[:, ch], V[:, m, h, :],
                                             eT[:, ch], start=(m == 0),
                                             stop=(m == nmb - 1))
                            nc.tensor.matmul(R[:, ch], hh2col[:, m:m + 1],
                                             e2T[:, ch], start=(m == 0),
                                             stop=(m == nmb - 1))
                    # gather per-head rows; engines need 32-aligned partition
                    # bases so stage + SBUF->SBUF/DRAM DMA
                    cstg = mpool.tile([DK, N], f32, tag="cstg", bufs=1)
                    nc.vector.tensor_copy(cstg[:], C[0:DK, :])
                    nc.sync.dma_start(ctx_d.ap()[h * DK:(h + 1) * DK, :],
                                      cstg[:])
                    stg_s = mpool.tile([1, N], f32, tag="stg")
                    nc.vector.tensor_copy(stg_s[:], C[DK:DK + 1, :])
                    nc.sync.dma_start(srows[h:h + 1, :], stg_s[:])
                    stg_v = mpool.tile([1, N], f32, tag="stg")
                    nc.vector.tensor_copy(stg_v[:], R[0:1, :])
                    nc.sync.dma_start(varows[h:h + 1, :], stg_v[:])

            # ---- P3: epilogue ----
            nc.vector.reciprocal(rinv[:], srows[:])
            nc.vector.tensor_tensor(rinv2[:], rinv[:], rinv[:], Alu.mult)
            nc.vector.tensor_tensor(vaq[0:NH, :], varows[:], rinv2[:],
                                    Alu.mult)

            WoT = bpool.tile([P, nod, D], f32, tag="tH")
            nc.sync.dma_start(
                WoT[:], WoT_d.ap().rearrange("(k p) n -> p k n", p=P))
            ctxNT = bpool.tile([P, nod, N], f32, tag="tE")

            for k in range(nod):
                cl = mpool.tile([P, N], f32, tag="cl")
                nc.sync.dma_start(cl[:], ctx_d.ap()[k * P:(k + 1) * P, :])
                rp = mpool.tile([2, N], f32, tag="rp", bufs=1)
                nc.sync.dma_start(rp[:], rinv[2 * k:2 * k + 2, :])
                b = psA.tile([P, N], f32, tag="A")
                for c in range(nch):
                    ch = slice(c * chn, (c + 1) * chn)
                    nc.tensor.matmul(b[:, ch], selrep[0:2, :], rp[:, ch],
                                     start=True, stop=True)
                nc.vector.tensor_tensor(ctxNT[:, k, :], cl[:], b[:],
                                        Alu.mult)

            qp = psB.tile([1, N], f32, tag="B")
            for k in range(nod):
                cn2 = mpool.tile([P, N], f32, tag="cn2")
                nc.vector.tensor_tensor(cn2[:], ctxNT[:, k, :],
                                        ctxNT[:, k, :], Alu.mult)
                for c in range(nch):
                    ch = slice(c * chn, (c + 1) * chn)
                    nc.tensor.matmul(qp[:, ch], onescol[:], cn2[:, ch],
                                     start=(k == 0), stop=(k == nod - 1))

            stg_q = mpool.tile([1, N], f32, tag="stg")
            nc.vector.tensor_scalar(stg_q[:], qp[:], cfg.co2,
                                    cfg.cbo, Alu.mult, Alu.add)
            nc.sync.dma_start(vaq[NH:NH + 1, :], stg_q[:])

            for t in range(nmb):
                tp = psB.tile([P, NH + 1], f32, tag="B")
                nc.tensor.transpose(tp[:, 0:NH + 1],
                                    vaq[:, t * P:(t + 1) * P],
                                    I128[0:NH + 1, 0:NH + 1])
                vt = mpool.tile([P, NH + 1], f32, tag="vt")
                nc.vector.tensor_copy(vt[:], tp[:])
                vcomb = mpool.tile([P, NH], f32, tag="vc")
                nc.vector.tensor_scalar(vcomb[:], vt[:, 0:NH],
                                        vt[:, NH:NH + 1], None, Alu.add)
                ov = mpool.tile([P, NH, DK], f32, tag="out", bufs=3)
                nc.vector.tensor_copy(
                    ov[:],
                    vcomb[:].unsqueeze(2).to_broadcast([P, NH, DK]))
                nc.sync.dma_start(
                    ov_d.ap()[t * P:(t + 1) * P, :],
                    ov[:].rearrange("p h d -> p (h d)"))

                mp = psA.tile([P, D], f32, tag="A")
                for c in range(ndch):
                    ch = slice(c * chd, (c + 1) * chd)
                    nc.tensor.matmul(mp[:, ch], onesrow[:], borow[:, ch],
                                     start=True, stop=False)
                for k in range(nod):
                    for c in range(ndch):
                        ch = slice(c * chd, (c + 1) * chd)
                        nc.tensor.matmul(mp[:, ch],
                                         ctxNT[:, k, t * P:(t + 1) * P],
                                         WoT[:, k, ch], start=False,
                                         stop=(k == nod - 1))
                om = mpool.tile([P, D], f32, tag="out", bufs=3)
                nc.vector.tensor_copy(om[:], mp[:])
                nc.sync.dma_start(om_d.ap()[t * P:(t + 1) * P, :], om[:])

    nc.compile()
    return nc


_CACHE = {}


def _get_program(cfg: Cfg):
    k = cfg.key() + (cfg.co2, cfg.cbo)
    if k not in _CACHE:
        _CACHE[k] = build_program(cfg)
    return _CACHE[k]


def _prep_core_inputs(cfg: Cfg, Hb, xb, WqT, WkT, WvT, WoT, bqs, bk, bv, bo,
                      cv2, cbv, consts):
    N, D, NH, draw, nmb, nod = cfg.N, cfg.D, cfg.NH, cfg.draw, cfg.nmb, cfg.nod
    xT = np.ascontiguousarray(xb.T)
    x2 = (xb * xb).sum(-1).astype(np.float32)
    hh = (Hb * Hb).sum(-1).astype(np.float32)
    hh2 = (cv2 * hh + cbv).astype(np.float32)
    m = {
        "HT": np.ascontiguousarray(Hb.T),
        "xT": xT,
        "xTm2": np.ascontiguousarray(-2.0 * xT),
        "x2row": x2[None, :],
        "x2col": np.ascontiguousarray(x2.reshape(nmb, P).T),
        "hh2col": np.ascontiguousarray(hh2.reshape(nmb, P).T),
        "WqT": WqT, "WkT": WkT, "WvT": WvT, "WoT": WoT,
        "bqc": np.ascontiguousarray(bqs.reshape(nod, P).T),
        "bkc": np.ascontiguousarray(bk.reshape(nod, P).T),
        "bvrow": bv[None, :],
        "borow": bo[None, :],
    }
    m.update(consts)
    return m


def _numpy_reference(H, x_raw, Wq_mu, Wq_rho, bq_mu, bq_rho, Wk_mu, Wk_rho,
                     bk_mu, bk_rho, Wv_mu, Wv_rho, bv_mu, bv_rho, Wo_mu,
                     Wo_rho, bo_mu, bo_rho, log_sigma_f, log_length):
    """Pure-numpy fallback (matches reference.py)."""
    B, N, D = H.shape
    NH = log_sigma_f.shape[0]
    DK = D // NH
    SCALE = 1.0 / np.sqrt(DK)

    def sp(v):
        return _softplus(v)

    Q = H @ Wq_mu.T + bq_mu
    K = H @ Wk_mu.T + bk_mu
    wsv = sp(Wv_rho); bsv = sp(bv_rho)
    Vm = H @ Wv_mu.T + bv_mu
    Vv = (H * H) @ (wsv * wsv).T + bsv * bsv

    def sh(t):
        return t.reshape(B, N, NH, DK).transpose(0, 2, 1, 3)

    Q, K, Vm, Vv = sh(Q), sh(K), sh(Vm), sh(Vv)
    scores = np.einsum("bhnd,bhmd->bhnm", Q, K) * SCALE
    s2 = np.exp(2.0 * log_sigma_f)
    l2 = np.exp(2.0 * log_length)
    x2 = (x_raw * x_raw).sum(-1)
    d2 = np.maximum(x2[:, :, None] + x2[:, None, :]
                    - 2.0 * np.einsum("bnd,bmd->bnm", x_raw, x_raw), 0.0)
    rbf = s2[None, :, None, None] * np.exp(
        -d2[:, None, :, :] / (2.0 * l2[None, :, None, None]))
    scores = scores + rbf
    scores -= scores.max(-1, keepdims=True)
    e = np.exp(scores)
    attn = e / e.sum(-1, keepdims=True)
    ctx = np.einsum("bhnm,bhmd->bhnd", attn, Vm)
    ctx = ctx.transpose(0, 2, 1, 3).reshape(B, N, D)
    var_attn = np.einsum("bhnm,bhmd->bhnd", attn * attn, Vv)
    var_attn = var_attn.transpose(0, 2, 1, 3).reshape(B, N, D)
    wso = sp(Wo_rho); bso = sp(bo_rho)
    out_mean = ctx @ Wo_mu.T + bo_mu
    out_var = (ctx * ctx) @ (wso * wso).T + bso * bso + var_attn
    return out_mean.astype(np.float32), out_var.astype(np.float32)


def kernel(**inputs):
    H = np.asarray(inputs["H"], np.float32)
    x_raw = np.asarray(inputs["x_raw"], np.float32)
    B, N, D = H.shape
    draw = x_raw.shape[-1]
    log_sigma_f = np.asarray(inputs["log_sigma_f"], np.float32)
    log_length = np.asarray(inputs["log_length"], np.float32)
    NH = log_sigma_f.shape[0]

    Wv_rho = np.asarray(inputs["Wv_rho"], np.float32)
    Wo_rho = np.asarray(inputs["Wo_rho"], np.float32)
    bv_rho = np.asarray(inputs["bv_rho"], np.float32)
    bo_rho = np.asarray(inputs["bo_rho"], np.float32)

    rank1_ok = (np.ptp(Wv_rho) == 0.0 and np.ptp(Wo_rho) == 0.0
                and np.ptp(bv_rho) == 0.0 and np.ptp(bo_rho) == 0.0
                and B == 8 and N % P == 0 and D % P == 0
                and D // NH == 64 and N % CH == 0)
    if not rank1_ok:
        return _numpy_reference(
            H, x_raw, *[np.asarray(inputs[k], np.float32) for k in (
                "Wq_mu", "Wq_rho", "bq_mu", "bq_rho",
                "Wk_mu", "Wk_rho", "bk_mu", "bk_rho",
                "Wv_mu", "Wv_rho", "bv_mu", "bv_rho",
                "Wo_mu", "Wo_rho", "bo_mu", "bo_rho")],
            log_sigma_f, log_length)

    cv2 = float(_softplus(Wv_rho.flat[0]) ** 2)
    cbv = float(_softplus(bv_rho.flat[0]) ** 2)
    co2 = float(_softplus(Wo_rho.flat[0]) ** 2)
    cbo = float(_softplus(bo_rho.flat[0]) ** 2)

    inv2l2 = 1.0 / (2.0 * np.exp(2.0 * log_length))
    logs2 = 2.0 * log_sigma_f
    shared = bool(np.ptp(inv2l2) == 0.0 and np.ptp(logs2) == 0.0)

    cfg = Cfg(N=N, D=D, NH=NH, draw=draw, shared_rbf=shared,
              inv2l2=(float(inv2l2[0]) if shared else inv2l2.tolist()),
              logs2=(float(logs2[0]) if shared else logs2.tolist()),
              e2_act_heads=tuple(range(0, NH, 2)))
    cfg.co2, cfg.cbo = co2, cbo
    nc = _get_program(cfg)

    SCALE = np.float32(1.0 / np.sqrt(D // NH))
    WqT = np.ascontiguousarray(np.asarray(inputs["Wq_mu"], np.float32).T
                               * SCALE)
    WkT = np.ascontiguousarray(np.asarray(inputs["Wk_mu"], np.float32).T)
    WvT = np.ascontiguousarray(np.asarray(inputs["Wv_mu"], np.float32).T)
    WoT = np.ascontiguousarray(np.asarray(inputs["Wo_mu"], np.float32).T)
    bqs = np.asarray(inputs["bq_mu"], np.float32) * SCALE
    bk = np.asarray(inputs["bk_mu"], np.float32)
    bv = np.asarray(inputs["bv_mu"], np.float32)
    bo = np.asarray(inputs["bo_mu"], np.float32)

    selrep = np.zeros((NH, P), np.float32)
    for j in range(NH):
        selrep[j, (j % 2) * 64:(j % 2) * 64 + 64] = 1.0
    consts = {"I128": np.eye(P, dtype=np.float32), "selrep": selrep}

    in_maps = [
        _prep_core_inputs(cfg, H[b], x_raw[b], WqT, WkT, WvT, WoT,
                          bqs, bk, bv, bo, cv2, cbv, consts)
        for b in range(B)
    ]

    global _LAST_IN_MAPS
    _LAST_IN_MAPS = in_maps

    from concourse.bass_utils import run_bass_kernel_spmd
    res = run_bass_kernel_spmd(nc, in_maps, list(range(B)))

    out_mean = np.stack([res.results[b]["om"] for b in range(B)])
    out_var = np.stack([res.results[b]["ov"] for b in range(B)])
    return out_mean, out_var


if __name__ == "__main__":
    # tiny smoke: build only
    c = Cfg(N=256, D=256, NH=4, draw=16, shared_rbf=True, inv2l2=0.5,
            logs2=0.0)
    c.co2, c.cbo = 0.001, 0.001
    build_program(c)
    print("build OK")


# revision 17
# speedup vs baseline: 39.6597x; 39.6597x over previous
"""Bayesian multi-head attention on 8 Trainium2 NeuronCores (Bass/Tile).

Sharding: pure data-parallel over batch (B=8 -> 1 batch per core), weights
replicated. Per core, everything is computed in "transposed" layouts so the
matmul contraction dim always sits on SBUF partitions:

  QT/KT  [od, tok]   (W.T stationary over H.T)
  V      [tok, od]   interleaved with per-head ones columns (for softmax sums)
  scores [m, n] per head, PSUM-prefilled with the per-head RBF kernel via an
                 identity matmul so ACT computes exp(scores+rbf) from PSUM
  ctx.T  [d, n]      V-stationary over e-tiles, softmax sum rides aug column
  out    [tok, od]   ctx.T-stationary over Wo.T, final layout, no transposes

Runtime specialization (checked in kernel(), numpy fallback otherwise):
  Wv_rho/Wo_rho/b*_rho are constant -> the variational variance projections
  are rank-1: V_v[m,od] = cv2*hh[m]+cbv with hh=rowsum(H^2), so
  var_attn per head is a single weighted reduction of e^2 and
  out_var = co2*sum_D(ctx_n^2) + cbo + var_attn[head].
"""

import sys
sys.path.insert(0, "/opt/trn_rl_repo")

import numpy as np

P = 128
CH = 512  # matmul free-dim chunk (fp32 psum bank)


def _softplus(x):
    return np.log1p(np.exp(-np.abs(x))) + np.maximum(x, 0.0)


class Cfg:
    def __init__(self, N=1024, D=1024, NH=16, draw=16, shared_rbf=True,
                 inv2l2=None, logs2=None, e2_act_heads=()):
        self.N, self.D, self.NH, self.draw = N, D, NH, draw
        self.DK = D // NH
        assert self.DK == 64, "head pairing layout assumes d_k == 64"
        self.nmb = N // P      # token blocks (key side)
        self.nod = D // P      # feature blocks
        self.nch = max(1, N // CH)   # free chunks over tokens
        self.ndch = max(1, D // CH)  # free chunks over features
        self.chn = min(CH, N)
        self.chd = min(CH, D)
        self.shared_rbf = shared_rbf
        # per-head (or shared) rbf params, baked as immediates
        self.inv2l2 = inv2l2   # shared: float; else list[NH]
        self.logs2 = logs2
        self.e2_act_heads = set(e2_act_heads)  # heads whose e^2 runs on ACT

    def key(self):
        return (self.N, self.D, self.NH, self.draw, self.shared_rbf,
                tuple(np.atleast_1d(self.inv2l2).tolist()),
                tuple(np.atleast_1d(self.logs2).tolist()),
                tuple(sorted(self.e2_act_heads)))


def build_program(cfg: Cfg):
    import concourse.bacc as bacc
    import concourse.mybir as mybir
    import concourse.tile as tile

    f32 = mybir.dt.float32
    Alu = mybir.AluOpType
    Act = mybir.ActivationFunctionType
    N, D, NH, DK, draw = cfg.N, cfg.D, cfg.NH, cfg.DK, cfg.draw
    nmb, nod, nch, ndch, chn, chd = (cfg.nmb, cfg.nod, cfg.nch, cfg.ndch,
                                     cfg.chn, cfg.chd)

    nc = bacc.Bacc("TRN2", target_bir_lowering=False, debug=False,
                   num_devices=8)

    # ---- dram I/O ----
    f32r = mybir.dt.float32r
    dt_in = {}
    def din(name, shape, dt=f32):
        dt_in[name] = nc.dram_tensor(name, list(shape), dt,
                                     kind="ExternalInput")
        return dt_in[name]

    HT_d = din("HT", [D, N], f32r)
    xT_d = din("xT", [draw, N], f32r)
    xTm2_d = din("xTm2", [draw, N], f32r)
    x2row_d = din("x2row", [1, N], f32r)
    x2col_d = din("x2col", [P, nmb])
    hh2col_d = din("hh2col", [P, nmb], f32r)
    WqT_d = din("WqT", [D, D], f32r)
    WkT_d = din("WkT", [D, D], f32r)
    WvT_d = din("WvT", [D, D], f32r)
    WoT_d = din("WoT", [D, D], f32r)
    bqc_d = din("bqc", [P, nod])
    bkc_d = din("bkc", [P, nod])
    bvrow_d = din("bvrow", [1, D], f32r)
    borow_d = din("borow", [1, D], f32r)
    I128_d = din("I128", [P, P])
    I128r_d = din("I128r", [P, P], f32r)
    selrep_d = din("selrep", [NH, P], f32r)
    om_d = nc.dram_tensor("om", [N, D], f32, kind="ExternalOutput")
    ov_d = nc.dram_tensor("ov", [N, D], f32, kind="ExternalOutput")

    ctx_d = nc.dram_tensor("ctx_scratch", [D, N], f32)

    with tile.TileContext(nc) as tc:
        with tc.tile_pool(name="const", bufs=1) as cpool, \
             tc.tile_pool(name="big", bufs=1) as bpool, \
             tc.tile_pool(name="rows", bufs=1) as rpool, \
             tc.tile_pool(name="wstream", bufs=3) as wpool, \
             tc.tile_pool(name="estream", bufs=2) as epool, \
             tc.tile_pool(name="mstream", bufs=2) as mpool, \
             tc.tile_pool(name="psA", bufs=2, space="PSUM") as psA, \
             tc.tile_pool(name="psB", bufs=2, space="PSUM") as psB:

            # ---- constants / small inputs ----
            I128 = cpool.tile([P, P], f32)
            nc.sync.dma_start(I128[:], I128_d.ap())
            I128r = cpool.tile([P, P], f32r)
            nc.sync.dma_start(I128r[:], I128r_d.ap())
            selrep = cpool.tile([NH, P], f32r)
            nc.sync.dma_start(selrep[:], selrep_d.ap())
            xT = mpool.tile([draw, N], f32r, tag="cl")
            nc.sync.dma_start(xT[:], xT_d.ap())
            xTm2 = mpool.tile([draw, N], f32r, tag="cl")
            nc.sync.dma_start(xTm2[:], xTm2_d.ap())
            x2row = mpool.tile([1, N], f32r, tag="rp", bufs=1)
            nc.sync.dma_start(x2row[:], x2row_d.ap())
            x2col = cpool.tile([P, nmb], f32)
            nc.sync.dma_start(x2col[:], x2col_d.ap())
            hh2col = cpool.tile([P, nmb], f32r)
            nc.sync.dma_start(hh2col[:], hh2col_d.ap())
            bqc = cpool.tile([P, nod], f32)
            nc.sync.dma_start(bqc[:], bqc_d.ap())
            bkc = cpool.tile([P, nod], f32)
            nc.sync.dma_start(bkc[:], bkc_d.ap())
            bvrow = mpool.tile([1, D], f32r, tag="stg")
            nc.sync.dma_start(bvrow[:], bvrow_d.ap())
            borow = cpool.tile([1, D], f32r)
            nc.sync.dma_start(borow[:], borow_d.ap())
            onesrow = cpool.tile([1, P], f32r)
            nc.vector.memset(onesrow[:].bitcast(f32), 1.0)
            onescol = cpool.tile([P, 1], f32r)
            nc.vector.memset(onescol[:].bitcast(f32), 1.0)
            zcol = cpool.tile([P, 1], f32)
            nc.vector.memset(zcol[:], 0.0)

            # ---- big persistent tensors (tag-shared slots) ----
            # slot tH: HT (P1-P2) -> WoT (P3); slot tE: E/d2 (P0-P2) -> ctxNT
            HT = bpool.tile([P, nod, N], f32r, tag="tH")
            nc.sync.dma_start(
                HT[:], HT_d.ap().rearrange("(k p) n -> p k n", p=P))
            V = bpool.tile([P, nmb, NH, DK + 1], f32r, tag="tV")
            # shared-rbf: E tiles; general: d2 tiles
            Ed2 = bpool.tile([P, nmb, N], f32r, tag="tE")

            srows = rpool.tile([NH, N], f32, tag="srows")
            varows = rpool.tile([NH, N], f32)
            rinv = rpool.tile([NH, N], f32r)
            rinv2 = rpool.tile([NH, N], f32)
            vaq = rpool.tile([NH + 1, N], f32, tag="srows")

            # ---- P0: rbf groundwork ----
            if cfg.shared_rbf:
                biascol = cpool.tile([P, nmb], f32)
                nc.vector.tensor_scalar(
                    biascol[:], x2col[:], -cfg.inv2l2, cfg.logs2,
                    Alu.mult, Alu.add)
            else:
                logs2col = cpool.tile([P, NH], f32)
                for h in range(NH):
                    nc.vector.memset(logs2col[:, h:h + 1], float(cfg.logs2[h]))

            for m in range(nmb):
                g = psA.tile([P, N], f32, tag="A")
                for c in range(nch):
                    ch = slice(c * chn, (c + 1) * chn)
                    nc.tensor.matmul(g[:, ch], xTm2[:, m * P:(m + 1) * P],
                                     xT[:, ch], start=True, stop=False)
                    nc.tensor.matmul(g[:, ch], onesrow[:], x2row[:, ch],
                                     start=False, stop=True)
                if cfg.shared_rbf:
                    # E = exp(-(d2)/(2 l^2) + log s2);  d2 = g + x2col
                    nc.scalar.activation(Ed2[:, m, :], g[:], Act.Exp,
                                         bias=biascol[:, m:m + 1],
                                         scale=-cfg.inv2l2)
                else:
                    nc.vector.tensor_scalar(Ed2[:, m, :], g[:],
                                            x2col[:, m:m + 1], None, Alu.add)

            # ---- P1: V projection ([tok, od] layout, ones interleaved) ----
            for t in range(nmb):
                pv = psA.tile([P, D], f32, tag="A")
                for c in range(ndch):
                    ch = slice(c * chd, (c + 1) * chd)
                    nc.tensor.matmul(pv[:, ch], onesrow[:], bvrow[:, ch],
                                     start=True, stop=False)
                for k in range(nod):
                    wv = wpool.tile([P, D], f32r, tag="wv", bufs=2)
                    nc.sync.dma_start(wv[:], WvT_d.ap()[k * P:(k + 1) * P, :])
                    for c in range(ndch):
                        ch = slice(c * chd, (c + 1) * chd)
                        nc.tensor.matmul(pv[:, ch],
                                         HT[:, k, t * P:(t + 1) * P],
                                         wv[:, ch], start=False,
                                         stop=(k == nod - 1))
                nc.vector.tensor_copy(
                    V[:, t, :, 0:DK],
                    pv[:].rearrange("p (h d) -> p h d", h=NH))
                nc.vector.memset(V[:, t, :, DK:DK + 1].bitcast(f32), 1.0)

            # ---- P2: head pairs (Q/K projected per pair, streamed) ----
            for g in range(NH // 2):
                QT = mpool.tile([P, N], f32r, tag="qt")
                KT = mpool.tile([P, N], f32r, tag="kt")
                for (Wd, bc, dst) in ((WqT_d, bqc, QT), (WkT_d, bkc, KT)):
                    wt = wpool.tile([P, nod, P], f32r, tag="w", bufs=2)
                    nc.sync.dma_start(
                        wt[:],
                        Wd.ap()[:, g * P:(g + 1) * P].rearrange(
                            "(k p) m -> p k m", p=P))
                    pp = psA.tile([P, N], f32, tag="A")
                    for k in range(nod):
                        for c in range(nch):
                            ch = slice(c * chn, (c + 1) * chn)
                            nc.tensor.matmul(pp[:, ch], wt[:, k, :],
                                             HT[:, k, ch],
                                             start=(k == 0),
                                             stop=(k == nod - 1))
                    nc.vector.tensor_scalar(dst[:], pp[:],
                                            bc[:, g:g + 1], None, Alu.add)

                for h in (2 * g, 2 * g + 1):
                    off = (h % 2) * DK
                    C = psB.tile([DK + 1, N], f32, tag="B")
                    R = psB.tile([1, N], f32, tag="B")
                    for m in range(nmb):
                        s = psA.tile([P, N], f32, tag="A")
                        if cfg.shared_rbf:
                            Et = Ed2[:, m, :]
                        else:
                            Et = epool.tile([P, N], f32r, tag="rbf")
                            nc.scalar.activation(Et[:], Ed2[:, m, :], Act.Exp,
                                                 bias=logs2col[:, h:h + 1],
                                                 scale=-float(cfg.inv2l2[h]))
                            Et = Et[:]
                        for c in range(nch):
                            ch = slice(c * chn, (c + 1) * chn)
                            nc.tensor.matmul(s[:, ch], I128r[:], Et[:, ch],
                                             start=True, stop=False)
                            nc.tensor.matmul(
                                s[:, ch],
                                KT[off:off + DK, m * P:(m + 1) * P],
                                QT[off:off + DK, ch],
                                start=False, stop=True)
                        eT = epool.tile([P, N], f32r, tag="e")
                        nc.scalar.activation(eT[:], s[:], Act.Exp,
                                             bias=zcol[:, 0:1])
                        e2T = epool.tile([P, N], f32r, tag="e2")
                        if h in cfg.e2_act_heads:
                            nc.scalar.activation(e2T[:], s[:], Act.Exp,
                                                 bias=zcol[:, 0:1], scale=2.0)
                        else:
                            nc.vector.tensor_tensor(e2T[:], eT[:], eT[:],
                                                    Alu.mult)
                        for c in range(nch):
                            ch = slice(c * chn, (c + 1) * chn)
                            nc.tensor.matmul(C[:, ch], V[:, m, h, :],
                                             eT[:, ch], start=(m == 0),
                                             stop=(m == nmb - 1))
                            nc.tensor.matmul(R[:, ch], hh2col[:, m:m + 1],
                                             e2T[:, ch], start=(m == 0),
                                             stop=(m == nmb - 1))
                    # gather per-head rows; engines need 32-aligned partition
                    # bases so stage + SBUF->SBUF/DRAM DMA
                    cstg = mpool.tile([DK, N], f32, tag="cstg", bufs=1)
                    nc.vector.tensor_copy(cstg[:], C[0:DK, :])
                    nc.scalar.dma_start(ctx_d.ap()[h * DK:(h + 1) * DK, :],
                                        cstg[:])
                    stg_s = mpool.tile([1, N], f32, tag="stg")
                    nc.vector.tensor_copy(stg_s[:], C[DK:DK + 1, :])
                    nc.scalar.dma_start(srows[h:h + 1, :], stg_s[:])
                    stg_v = mpool.tile([1, N], f32, tag="stg")
                    nc.vector.tensor_copy(stg_v[:], R[0:1, :])
                    nc.scalar.dma_start(varows[h:h + 1, :], stg_v[:])

            # ---- P3: epilogue ----
            with nc.allow_low_precision(reason="fp32r normalize"):
                nc.vector.reciprocal(rinv[:], srows[:])
            nc.vector.tensor_tensor(rinv2[:], rinv[:], rinv[:], Alu.mult)
            nc.vector.tensor_tensor(vaq[0:NH, :], varows[:], rinv2[:],
                                    Alu.mult)

            WoT = bpool.tile([P, nod, D], f32r, tag="tH")
            nc.sync.dma_start(
                WoT[:], WoT_d.ap().rearrange("(k p) n -> p k n", p=P))
            ctxNT = bpool.tile([P, nod, N], f32r, tag="tE")

            for k in range(nod):
                cl = mpool.tile([P, N], f32, tag="cl")
                nc.sync.dma_start(cl[:], ctx_d.ap()[k * P:(k + 1) * P, :])
                rp = mpool.tile([2, N], f32r, tag="rp", bufs=1)
                nc.sync.dma_start(rp[:], rinv[2 * k:2 * k + 2, :])
                b = psA.tile([P, N], f32, tag="A")
                for c in range(nch):
                    ch = slice(c * chn, (c + 1) * chn)
                    nc.tensor.matmul(b[:, ch], selrep[0:2, :], rp[:, ch],
                                     start=True, stop=True)
                nc.vector.tensor_tensor(ctxNT[:, k, :], cl[:], b[:],
                                        Alu.mult)

            qp = psB.tile([1, N], f32, tag="B")
            for k in range(nod):
                cn2 = mpool.tile([P, N], f32r, tag="cn2", bufs=1)
                nc.vector.tensor_tensor(cn2[:], ctxNT[:, k, :],
                                        ctxNT[:, k, :], Alu.mult)
                for c in range(nch):
                    ch = slice(c * chn, (c + 1) * chn)
                    nc.tensor.matmul(qp[:, ch], onescol[:], cn2[:, ch],
                                     start=(k == 0), stop=(k == nod - 1))

            stg_q = mpool.tile([1, N], f32, tag="stg")
            nc.vector.tensor_scalar(stg_q[:], qp[:], cfg.co2,
                                    cfg.cbo, Alu.mult, Alu.add)
            nc.scalar.dma_start(vaq[NH:NH + 1, :], stg_q[:])

            for t in range(nmb):
                tp = psB.tile([P, NH + 1], f32, tag="B")
                nc.tensor.transpose(tp[:, 0:NH + 1],
                                    vaq[:, t * P:(t + 1) * P],
                                    I128[0:NH + 1, 0:NH + 1])
                vt = mpool.tile([P, NH + 1], f32, tag="vt")
                nc.vector.tensor_copy(vt[:], tp[:])
                vcomb = mpool.tile([P, NH], f32, tag="vc")
                nc.vector.tensor_scalar(vcomb[:], vt[:, 0:NH],
                                        vt[:, NH:NH + 1], None, Alu.add)
                ov = mpool.tile([P, NH, DK], f32, tag="out", bufs=3)
                nc.vector.tensor_copy(
                    ov[:],
                    vcomb[:].unsqueeze(2).to_broadcast([P, NH, DK]))
                nc.scalar.dma_start(
                    ov_d.ap()[t * P:(t + 1) * P, :],
                    ov[:].rearrange("p h d -> p (h d)"))

                mp = psA.tile([P, D], f32, tag="A")
                for c in range(ndch):
                    ch = slice(c * chd, (c + 1) * chd)
                    nc.tensor.matmul(mp[:, ch], onesrow[:], borow[:, ch],
                                     start=True, stop=False)
                for k in range(nod):
                    for c in range(ndch):
                        ch = slice(c * chd, (c + 1) * chd)
                        nc.tensor.matmul(mp[:, ch],
                                         ctxNT[:, k, t * P:(t + 1) * P],
                                         WoT[:, k, ch], start=False,
                                         stop=(k == nod - 1))
                om = mpool.tile([P, D], f32, tag="out", bufs=3)
                nc.vector.tensor_copy(om[:], mp[:])
                nc.scalar.dma_start(om_d.ap()[t * P:(t + 1) * P, :], om[:])

    nc.compile()
    return nc


_CACHE = {}


def _get_program(cfg: Cfg):
    k = cfg.key() + (cfg.co2, cfg.cbo)
    if k not in _CACHE:
        _CACHE[k] = build_program(cfg)
    return _CACHE[k]


def _prep_core_inputs(cfg: Cfg, Hb, xb, WqT, WkT, WvT, WoT, bqs, bk, bv, bo,
                      cv2, cbv, consts):
    N, D, NH, draw, nmb, nod = cfg.N, cfg.D, cfg.NH, cfg.draw, cfg.nmb, cfg.nod
    xT = np.ascontiguousarray(xb.T)
    x2 = (xb * xb).sum(-1).astype(np.float32)
    hh = (Hb * Hb).sum(-1).astype(np.float32)
    hh2 = (cv2 * hh + cbv).astype(np.float32)
    m = {
        "HT": np.ascontiguousarray(Hb.T),
        "xT": xT,
        "xTm2": np.ascontiguousarray(-2.0 * xT),
        "x2row": x2[None, :],
        "x2col": np.ascontiguousarray(x2.reshape(nmb, P).T),
        "hh2col": np.ascontiguousarray(hh2.reshape(nmb, P).T),
        "WqT": WqT, "WkT": WkT, "WvT": WvT, "WoT": WoT,
        "bqc": np.ascontiguousarray(bqs.reshape(nod, P).T),
        "bkc": np.ascontiguousarray(bk.reshape(nod, P).T),
        "bvrow": bv[None, :],
        "borow": bo[None, :],
    }
    m.update(consts)
    return m


def _numpy_reference(H, x_raw, Wq_mu, Wq_rho, bq_mu, bq_rho, Wk_mu, Wk_rho,
                     bk_mu, bk_rho, Wv_mu, Wv_rho, bv_mu, bv_rho, Wo_mu,
                     Wo_rho, bo_mu, bo_rho, log_sigma_f, log_length):
    """Pure-numpy fallback (matches reference.py)."""
    B, N, D = H.shape
    NH = log_sigma_f.shape[0]
    DK = D // NH
    SCALE = 1.0 / np.sqrt(DK)

    def sp(v):
        return _softplus(v)

    Q = H @ Wq_mu.T + bq_mu
    K = H @ Wk_mu.T + bk_mu
    wsv = sp(Wv_rho); bsv = sp(bv_rho)
    Vm = H @ Wv_mu.T + bv_mu
    Vv = (H * H) @ (wsv * wsv).T + bsv * bsv

    def sh(t):
        return t.reshape(B, N, NH, DK).transpose(0, 2, 1, 3)

    Q, K, Vm, Vv = sh(Q), sh(K), sh(Vm), sh(Vv)
    scores = np.einsum("bhnd,bhmd->bhnm", Q, K) * SCALE
    s2 = np.exp(2.0 * log_sigma_f)
    l2 = np.exp(2.0 * log_length)
    x2 = (x_raw * x_raw).sum(-1)
    d2 = np.maximum(x2[:, :, None] + x2[:, None, :]
                    - 2.0 * np.einsum("bnd,bmd->bnm", x_raw, x_raw), 0.0)
    rbf = s2[None, :, None, None] * np.exp(
        -d2[:, None, :, :] / (2.0 * l2[None, :, None, None]))
    scores = scores + rbf
    scores -= scores.max(-1, keepdims=True)
    e = np.exp(scores)
    attn = e / e.sum(-1, keepdims=True)
    ctx = np.einsum("bhnm,bhmd->bhnd", attn, Vm)
    ctx = ctx.transpose(0, 2, 1, 3).reshape(B, N, D)
    var_attn = np.einsum("bhnm,bhmd->bhnd", attn * attn, Vv)
    var_attn = var_attn.transpose(0, 2, 1, 3).reshape(B, N, D)
    wso = sp(Wo_rho); bso = sp(bo_rho)
    out_mean = ctx @ Wo_mu.T + bo_mu
    out_var = (ctx * ctx) @ (wso * wso).T + bso * bso + var_attn
    return out_mean.astype(np.float32), out_var.astype(np.float32)


def kernel(**inputs):
    H = np.asarray(inputs["H"], np.float32)
    x_raw = np.asarray(inputs["x_raw"], np.float32)
    B, N, D = H.shape
    draw = x_raw.shape[-1]
    log_sigma_f = np.asarray(inputs["log_sigma_f"], np.float32)
    log_length = np.asarray(inputs["log_length"], np.float32)
    NH = log_sigma_f.shape[0]

    Wv_rho = np.asarray(inputs["Wv_rho"], np.float32)
    Wo_rho = np.asarray(inputs["Wo_rho"], np.float32)
    bv_rho = np.asarray(inputs["bv_rho"], np.float32)
    bo_rho = np.asarray(inputs["bo_rho"], np.float32)

    rank1_ok = (np.ptp(Wv_rho) == 0.0 and np.ptp(Wo_rho) == 0.0
                and np.ptp(bv_rho) == 0.0 and np.ptp(bo_rho) == 0.0
                and B == 8 and N % P == 0 and D % P == 0
                and D // NH == 64 and N % CH == 0)
    if not rank1_ok:
        return _numpy_reference(
            H, x_raw, *[np.asarray(inputs[k], np.float32) for k in (
                "Wq_mu", "Wq_rho", "bq_mu", "bq_rho",
                "Wk_mu", "Wk_rho", "bk_mu", "bk_rho",
                "Wv_mu", "Wv_rho", "bv_mu", "bv_rho",
                "Wo_mu", "Wo_rho", "bo_mu", "bo_rho")],
            log_sigma_f, log_length)

    cv2 = float(_softplus(Wv_rho.flat[0]) ** 2)
    cbv = float(_softplus(bv_rho.flat[0]) ** 2)
    co2 = float(_softplus(Wo_rho.flat[0]) ** 2)
    cbo = float(_softplus(bo_rho.flat[0]) ** 2)

    inv2l2 = 1.0 / (2.0 * np.exp(2.0 * log_length))
    logs2 = 2.0 * log_sigma_f
    shared = bool(np.ptp(inv2l2) == 0.0 and np.ptp(logs2) == 0.0)

    cfg = Cfg(N=N, D=D, NH=NH, draw=draw, shared_rbf=shared,
              inv2l2=(float(inv2l2[0]) if shared else inv2l2.tolist()),
              logs2=(float(logs2[0]) if shared else logs2.tolist()),
              e2_act_heads=tuple(range(0, NH, 2)))
    cfg.co2, cfg.cbo = co2, cbo
    nc = _get_program(cfg)

    SCALE = np.float32(1.0 / np.sqrt(D // NH))
    WqT = np.ascontiguousarray(np.asarray(inputs["Wq_mu"], np.float32).T
                               * SCALE)
    WkT = np.ascontiguousarray(np.asarray(inputs["Wk_mu"], np.float32).T)
    WvT = np.ascontiguousarray(np.asarray(inputs["Wv_mu"], np.float32).T)
    WoT = np.ascontiguousarray(np.asarray(inputs["Wo_mu"], np.float32).T)
    bqs = np.asarray(inputs["bq_mu"], np.float32) * SCALE
    bk = np.asarray(inputs["bk_mu"], np.float32)
    bv = np.asarray(inputs["bv_mu"], np.float32)
    bo = np.asarray(inputs["bo_mu"], np.float32)

    selrep = np.zeros((NH, P), np.float32)
    for j in range(NH):
        selrep[j, (j % 2) * 64:(j % 2) * 64 + 64] = 1.0
    consts = {"I128": np.eye(P, dtype=np.float32),
              "I128r": np.eye(P, dtype=np.float32), "selrep": selrep}

    in_maps = [
        _prep_core_inputs(cfg, H[b], x_raw[b], WqT, WkT, WvT, WoT,
                          bqs, bk, bv, bo, cv2, cbv, consts)
        for b in range(B)
    ]

    global _LAST_IN_MAPS
    _LAST_IN_MAPS = in_maps

    from concourse.bass_utils import run_bass_kernel_spmd
    res = run_bass_kernel_spmd(nc, in_maps, list(range(B)))

    out_mean = np.stack([res.results[b]["om"] for b in range(B)])
    out_var = np.stack([res.results[b]["ov"] for b in range(B)])
    return out_mean, out_var


if __name__ == "__main__":
    # tiny smoke: build only
    c = Cfg(N=256, D=256, NH=4, draw=16, shared_rbf=True, inv2l2=0.5,
            logs2=0.0)
    c.co2, c.cbo = 0.001, 0.001
    build_program(c)
    print("build OK")
